# revision 22
# baseline (speedup 1.0000x reference)
"""Trainium2 Bass kernel for nn_CNNPolicyHead (KataGo-style CNN policy head).

Contract: kernel(**inputs) takes FULL unsharded inputs (as produced by the
reference setup_inputs) and returns the FULL output [1024, 6, 362] fp32.

Strategy: pure data parallel over 8 NeuronCores — batch N=1024 sharded 128
per core; all params replicated.  The problem is memory-bound on the x load
(71 MB fp32 per core), so x is cast to bf16 and repacked on the host into a
DMA-optimal layout (fully contiguous per SBUF partition, 8 items per 2.2 MB
transfer).  bf16 keeps the matmul error ~1e-3 relative, well inside the
2e-2 gate.

Per core, per batch item i (HW = 19*19 = 361, padded to HWP = 362):

  PE:   psum1[112,362]  = 3 accumulating bf16 matmuls (rows 0:48 conv1p,
        rows 64:112 conv1g; the 16-row gap keeps the conv1g slice
        32-partition aligned), then a tiny fp32 matmul overwrites psum
        column 361 with -1000 so the pad column is inert under relu for
        both the sum and the max that follow.
  ACT:  outg(bf16) = relu(psum1[64:112] + beta_g), fused accum -> Gsum[:,i]
  DVE:  Gmax[:,i] = rowmax(outg)   (bf16 2x mode, even width 362)
  per group of 4 items:
        Gmean/Gmoff = Gsum * invms/offinv (bf16); 6 tiny bf16 matmuls
        produce bias = wlg.T@gpooled + beta_2 and passrelu.
  DVE:  outp(bf16) = relu(psum1[0:48] + bias_i)
  PE:   psum2 pair tile [2, 2, 512] (spans exactly 2 PSUM banks, one item
        per bank): w2t.T @ outp into [:, j, 0:362]; fp32 1-col matmul
        overwrites col 361 with the pass logits wp2.T @ passrelu_i
  ACT:  stage[2, pair, :] <- psum2 pair (one copy per TWO items, halving
        the per-instruction PSUM-access bubble); one gpsimd out-DMA per
        group of 4.

mask is all-ones by construction (spec fill=ones); mask_sum_hw is consumed
as data via host-prepped per-item scalars (invms, offinv).
"""
import sys

if "/opt/trn_rl_repo" not in sys.path:
    sys.path.insert(0, "/opt/trn_rl_repo")

import numpy as np
import ml_dtypes

N, C_IN, HW = 1024, 384, 361
HWP = 362  # even-padded moving width (bf16 rows stay 4B aligned)
C_P1, C_G1 = 48, 48
N_CORES = 8
NPC = N // N_CORES   # items per core
DMAB = 8             # items per x DMA batch
GROUP = 4            # items per processing group (PSUM-lifetime bound)

# pk16 (bf16) column layout: w2t 0:2 | wlg 2:146 | wp 146:290
PK16_COLS = 290
# pk32 (fp32) column layout: wp2t 0:2 | betag 2 | beta2 3 | bpass 4 |
#   invms 5:133 | offinv 133:261 | neg(-1000 row) 261:373 | ones 373
PK32_COLS = 374

_cache = {}


def _build(npc=NPC, repeat=1, xbufs=3, gbufs=3, pbufs=3, stbufs=2,
           ps1b=5, copy_dve_every=0, ablate=None):
    import concourse.bacc as bacc
    import concourse.mybir as mybir
    import concourse.tile as tile

    f32 = mybir.dt.float32
    bf16 = mybir.dt.bfloat16
    AF = mybir.ActivationFunctionType
    ALU = mybir.AluOpType
    AX = mybir.AxisListType

    nbatch = npc // DMAB
    sub_per_batch = DMAB // GROUP
    nc = bacc.Bacc("TRN2", target_bir_lowering=False, debug=False)

    x_d = nc.dram_tensor("x", [nbatch, 128, DMAB, 3, HWP], bf16,
                         kind="ExternalInput")
    w1_d = nc.dram_tensor("w1", [128, 3, 112], bf16, kind="ExternalInput")
    pk16_d = nc.dram_tensor("pk16", [48, PK16_COLS], bf16,
                            kind="ExternalInput")
    pk32_d = nc.dram_tensor("pk32", [48, PK32_COLS], f32,
                            kind="ExternalInput")
    out_d = nc.dram_tensor("out", [npc, 2, HWP], f32, kind="ExternalOutput")

    with tile.TileContext(nc) as tc:
        with (
            tc.tile_pool(name="const", bufs=1) as cpool,
            tc.tile_pool(name="x", bufs=xbufs) as xpool,
            tc.tile_pool(name="outg", bufs=gbufs) as gpool,
            tc.tile_pool(name="outp", bufs=pbufs) as ppool,
            tc.tile_pool(name="grp", bufs=2) as bgpool,
            tc.tile_pool(name="stage", bufs=stbufs) as stpool,
            tc.tile_pool(name="ps1", bufs=ps1b, space="PSUM") as ps1,
            tc.tile_pool(name="pssm", bufs=1, space="PSUM") as pssm,
            tc.tile_pool(name="pspair", bufs=1, space="PSUM") as pspair,
        ):
            w1_sb = cpool.tile([128, 3, 112], bf16)
            pk16 = cpool.tile([48, PK16_COLS], bf16)
            pk32 = cpool.tile([48, PK32_COLS], f32)
            Gsum = cpool.tile([48, npc], f32)
            Gmean = cpool.tile([48, npc], bf16)
            Gmoff = cpool.tile([48, npc], bf16)
            Gmax = cpool.tile([48, npc], bf16)

            nc.sync.dma_start(w1_sb[:], w1_d.ap()[:])
            nc.sync.dma_start(pk16[:], pk16_d.ap()[:])
            nc.sync.dma_start(pk32[:], pk32_d.ap()[:])

            w2t_ap = pk16[:, 0:2]
            betag_ap = pk32[:, 2:3]
            beta2_ap = pk32[:, 3:4]
            bpass_ap = pk32[:, 4:5]
            wp2t_ap = pk32[:, 0:2]
            neg_ap = pk32[0:1, 261:373]
            ones_ap = pk32[0:1, 373:374]

            def make_phase_bc(c0, p_tiles):
                """Group tail (pooled linears + per-item conv-out) as a list
                of emit-chunks, interleaved into the NEXT group's phase A so
                the bias dependency chain hides behind conv1/relu work."""
                c1 = c0 + GROUP

                def em_gmm():
                    nc.vector.tensor_tensor(
                        Gmean[:, c0:c1], Gsum[:, c0:c1],
                        pk32[:, 5 + c0:5 + c1], op=ALU.mult,
                    )
                    nc.vector.tensor_tensor(
                        Gmoff[:, c0:c1], Gsum[:, c0:c1],
                        pk32[:, 133 + c0:133 + c1], op=ALU.mult,
                    )

                state = {}

                def em_linmm():
                    psum_lin = pssm.tile([48, GROUP], f32, tag="sm")
                    for b, Gblk in enumerate((Gmean, Gmoff, Gmax)):
                        nc.tensor.matmul(
                            psum_lin[:], pk16[:, 2 + 48 * b:50 + 48 * b],
                            Gblk[:, c0:c1], start=(b == 0), stop=(b == 2),
                        )
                    bias_grp = bgpool.tile([48, GROUP], f32, tag="bias")
                    nc.vector.tensor_scalar(
                        bias_grp[:], psum_lin[:], beta2_ap, None, op0=ALU.add
                    )
                    state["bias"] = bias_grp

                def em_passmm():
                    psum_pass = pssm.tile([48, GROUP], f32, tag="sm")
                    for b, Gblk in enumerate((Gmean, Gmoff, Gmax)):
                        nc.tensor.matmul(
                            psum_pass[:], pk16[:, 146 + 48 * b:194 + 48 * b],
                            Gblk[:, c0:c1], start=(b == 0), stop=(b == 2),
                        )
                    passrelu = bgpool.tile([48, GROUP], f32, tag="prelu")
                    nc.vector.tensor_scalar(
                        passrelu[:], psum_pass[:], bpass_ap, 0.0,
                        op0=ALU.add, op1=ALU.max,
                    )
                    state["prelu"] = passrelu
                    state["stage"] = stpool.tile(
                        [2, GROUP, HWP], f32, tag="stage", name="stage")

                def em_pair(pp):
                    def emit():
                        psum2 = pspair.tile([2, 2, 512], f32, tag="pair")
                        for jj in range(2):
                            ii = 2 * pp + jj
                            outp = ppool.tile([48, HWP], bf16, tag="outp")
                            nc.vector.tensor_scalar(
                                outp[:], p_tiles[ii][0:48, :],
                                state["bias"][:, ii:ii + 1], 0.0,
                                op0=ALU.add, op1=ALU.max,
                            )
                            nc.tensor.matmul(
                                psum2[:, jj, 0:HWP], w2t_ap, outp[:],
                                start=True, stop=True, skip_group_check=True,
                            )
                            nc.tensor.matmul(
                                psum2[:, jj, HW:HWP], wp2t_ap,
                                state["prelu"][:, ii:ii + 1],
                                start=True, stop=True, skip_group_check=True,
                            )
                        # one PSUM->SBUF copy per TWO items
                        pair_i = (c0 // 2) + pp
                        if (copy_dve_every and pair_i % copy_dve_every
                                == copy_dve_every - 1):
                            nc.vector.tensor_copy(
                                state["stage"][:, 2 * pp:2 * pp + 2, :],
                                psum2[:, :, 0:HWP],
                            )
                        else:
                            nc.scalar.activation(
                                state["stage"][:, 2 * pp:2 * pp + 2, :],
                                psum2[:, :, 0:HWP], AF.Copy,
                            )
                    return emit

                def em_outdma():
                    nc.gpsimd.dma_start(
                        out_d.ap()[c0:c0 + GROUP, :, :].transpose([1, 0, 2]),
                        state["stage"][:],
                    )

                return ([em_gmm, em_linmm, em_passmm]
                        + [em_pair(pp) for pp in range(GROUP // 2)]
                        + [em_outdma])

            def drain(pending, k):
                for em in pending[:k]:
                    em()
                del pending[:k]

            pending = []
            for t in [tt for _ in range(repeat) for tt in range(nbatch)]:
                x_r = xpool.tile([128, DMAB, 3, HWP], bf16, tag="x")
                nc.sync.dma_start(x_r[:], x_d.ap()[t])
                for sub in range(sub_per_batch):
                    c0 = t * DMAB + sub * GROUP
                    if ablate == "dma":
                        continue
                    p_tiles = []
                    for ii in range(GROUP):
                        i = c0 + ii
                        xi = sub * GROUP + ii
                        psum1 = ps1.tile([112, HWP], f32, tag="ps1")
                        for k in range(3):
                            nc.tensor.matmul(
                                psum1[:], w1_sb[:, k, :], x_r[:, xi, k, :],
                                start=(k == 0), stop=(k == 2),
                            )
                        if ablate == "mm":
                            continue
                        # force pad col 361 to -1000: relu() of it is 0, so
                        # it is inert for both the accum-sum and the max
                        nc.tensor.matmul(
                            psum1[:, HW:HWP], neg_ap, ones_ap,
                            start=True, stop=True, skip_group_check=True,
                        )
                        if ii % 2 == 0:
                            outg = gpool.tile([48, 2, HWP], bf16, tag="outg")
                        nc.scalar.activation(
                            outg[:, ii % 2, :], psum1[64:112, :], AF.Relu,
                            bias=betag_ap, accum_out=Gsum[:, i:i + 1],
                        )
                        if ii % 2 == 1:
                            # one max-reduce per TWO items (axis X reduces
                            # the innermost dim only -> out [48, 2])
                            nc.vector.reduce_max(
                                Gmax[:, i - 1:i + 1], outg[:], axis=AX.X
                            )
                        p_tiles.append(psum1)
                        # interleave the previous group's tail AFTER this
                        # item's phase-A ops (ops emitted here have had a
                        # full phase-A of lag, so their deps are ready and
                        # the strict per-engine FIFO never stalls on them)
                        drain(pending, (len(pending) + GROUP - 2 - ii)
                              // (GROUP - ii) if ii < GROUP - 1 else 0)
                    if ablate == "phase1":
                        continue
                    pending.extend(make_phase_bc(c0, p_tiles))
            drain(pending, len(pending))

    nc.compile()
    return nc


def _to_bf16_u16(a):
    """fp32 -> bf16 bit pattern (round-half-up) as uint16, via integer ops —
    much faster than ml_dtypes astype on this host."""
    u = np.ascontiguousarray(a).view(np.uint32)
    return ((u + 0x8000) >> 16).astype(np.uint16)


def _prep_params(inputs):
    """Host-side packing of the small parameter tensors (shared by all
    cores).  Returns w1 (bf16), pk16 (bf16), pk32 (fp32)."""
    bf = ml_dtypes.bfloat16
    w_conv1p = np.asarray(inputs["w_conv1p"], np.float32)
    w_conv1g = np.asarray(inputs["w_conv1g"], np.float32)
    W1 = np.zeros((112, 384), np.float32)  # rows 48:64 stay zero (alignment)
    W1[0:48] = w_conv1p
    W1[64:112] = w_conv1g
    w1 = np.ascontiguousarray(
        W1.T.reshape(3, 128, 112).transpose(1, 0, 2)  # [128, 3, 112]
    ).astype(bf)

    pk16 = np.zeros((48, PK16_COLS), np.float32)
    pk16[:, 0:2] = np.asarray(inputs["w_conv2p"], np.float32).T
    pk16[:, 2:146] = np.asarray(inputs["w_linear_g"], np.float32).T.reshape(
        3, 48, 48).transpose(1, 0, 2).reshape(48, 144)
    pk16[:, 146:290] = np.asarray(
        inputs["w_linear_pass"], np.float32).T.reshape(
        3, 48, 48).transpose(1, 0, 2).reshape(48, 144)
    pk16 = pk16.astype(bf)

    ms = np.asarray(inputs["mask_sum_hw"], np.float32).reshape(-1)  # [N]
    invms = (1.0 / ms).astype(np.float32)
    offinv = (((np.sqrt(ms) - 14.0) / 10.0) / ms).astype(np.float32)

    pk32 = np.zeros((48, PK32_COLS), np.float32)
    pk32[:, 0:2] = np.asarray(inputs["w_linear_pass2"], np.float32).T
    pk32[:, 2] = np.asarray(inputs["beta_g"], np.float32)
    pk32[:, 3] = np.asarray(inputs["beta_2"], np.float32)
    pk32[:, 4] = np.asarray(inputs["b_linear_pass"], np.float32)
    pk32[:, 261:373] = -1000.0
    pk32[:, 373] = 1.0
    return w1, pk16, pk32, invms, offinv


def _prep_x_core(x_core_f32):
    """[NPC, 384, 361] fp32 -> [NPC//DMAB, 128, DMAB, 3, HWP] bf16 with a
    zero pad column, laid out so each DMA batch is contiguous per
    partition."""
    nb = x_core_f32.shape[0] // DMAB
    xb = _to_bf16_u16(x_core_f32)  # [npc, 384, 361] u16
    out = np.zeros((nb, 128, DMAB, 3, HWP), np.uint16)
    out[..., :HW] = (
        xb.reshape(nb, DMAB, 3, 128, HW).transpose(0, 3, 1, 2, 4)
    )
    return out.view(ml_dtypes.bfloat16)


def build_in_maps(inputs, npc=NPC):
    w1, pk16, pk32, invms, offinv = _prep_params(inputs)
    x = np.asarray(inputs["x"], np.float32).reshape(N, C_IN, HW)
    in_maps = []
    for c in range(N_CORES):
        s = slice(c * NPC, c * NPC + npc)
        pk32_c = pk32.copy()
        pk32_c[:, 5:5 + npc] = invms[s][None, :]
        pk32_c[:, 133:133 + npc] = offinv[s][None, :]
        in_maps.append({
            "x": _prep_x_core(x[s]),
            "w1": w1,
            "pk16": pk16,
            "pk32": pk32_c,
        })
    return in_maps


def kernel(**inputs) -> np.ndarray:
    from concourse import bass_utils

    if "nc" not in _cache:
        _cache["nc"] = _build()
    nc = _cache["nc"]

    in_maps = build_in_maps(inputs)
    res = bass_utils.run_bass_kernel_spmd(
        nc, in_maps, core_ids=list(range(N_CORES))
    )
    _cache["last_result"] = res

    full = np.zeros((N, 6, HW + 1), np.float32)
    for c in range(N_CORES):
        o = res.results[c]["out"]  # [NPC, 2, 362]
        full[c * NPC:(c + 1) * NPC, 0, :] = o[:, 0, :]
        full[c * NPC:(c + 1) * NPC, 5, :] = o[:, 1, :]
    return full
